# revision 35
# baseline (speedup 1.0000x reference)
"""Trainium2 Bass kernel for Bahdanau-style attention scoring.

Reference computation (per batch b):
    h_proj = hidden @ Wh.T + b_attn                  # [D]
    c_proj[s] = Wc @ context[b, s]                   # [S, D]
    scores[s] = v . tanh(h_proj + c_proj[s])         # [S]
    out[b] = softmax(where(mask==0, -inf, scores))   # [S]

Strategy: data-parallel over batch B across 8 NeuronCores (4 batches/core).
The roofline is the c_proj matmul: 1024 [128x128]x[128x512] fp16 matmuls per
core at the ~216ns/MM warm N=512 pace (512 cols / 2.4GHz + NX issue) =
~221us of TensorE streaming.  Everything else is engineered to stay off
that stream:

- context ships pre-cast to fp16 (32 MiB/core) in a window-contiguous
  layout [b, w, p, g, s_local] so every 1 MiB s-window is a single fully
  contiguous DMA (8 KiB per partition line), keeping descriptor count and
  SWDGE work minimal.
- c_proj is computed TRANSPOSED [s, d]: the context tile [e=128, s=128] is
  the stationary operand, wcT [e=128, d=512] the moving operand, so the
  v-dot after tanh is a free-axis mul+reduce on VectorE instead of TensorE
  mat-vecs.
- h_proj is folded into the context on the host, exactly: Wc has full row
  rank, so delta_b = Wc^T (Wc Wc^T)^{-1} h_proj_b satisfies
  Wc (x + delta_b) = c_proj + h_proj_b; shard prep adds delta_b[e] to
  batch b's context rows.  PSUM then holds tanh's full argument directly.
- mask + softmax run on the HOST after the gather (they are O(B*S), i.e.
  0.01% of the FLOPs): the device ships the raw score tiles [128, 32] per
  batch, so the exposed device tail after the last matmul is just the last
  tile's tanh/dot chain plus one 16 KiB DMA, not a serial softmax chain.

Startup: wcT's 8 chunks split across the sync+scalar HWDGE rings
(preamble-only: a dma_start blocked on a full ring would block ScalarE's
tanh chain behind it), window 0's chunks + window 1's halves + all later
windows stream on the gpsimd/SWDGE queue.  A ~3.7us continuous junk-MM
burst keeps the PE busy through the fill so the HAM clock-gate reaches
8/8 before the real stream starts (any >1.3us PE gap in the first ~30us
re-throttles the PE to half clock).  Windows 0 and 1 are consumed with
the contraction swept g-outermost across the 4 window PSUM banks, so each
matmul needs only the (wcT, ctx) chunk pair that has already arrived.

Per (b, s-tile of 128):
  - 8 accumulating matmuls (e-chunks) -> PSUM y.T [s=128, d=512]
  - ScalarE tanh (PSUM -> SBUF fp16)
  - VectorE (sim * v_bcast) at fp16 2x pace, then free-axis reduce_sum ->
    one column of the per-batch scores tile [128, 32]  (s = tile*128 + p)
The very last s-tile runs as two d-half matmul groups so its chain
pipelines with the second half's matmuls.
"""

import numpy as np

import concourse.bacc as bacc
import concourse.mybir as mybir
from concourse.tile import TileContext
from concourse.bass_utils import run_bass_kernel_spmd

B, S, E, D = 32, 4096, 1024, 512
NCORES = 8
BL = B // NCORES  # batches per core

F32 = mybir.dt.float32
F16 = mybir.dt.float16

JUNK = 72  # PE warm-up matmuls: ~3.7us continuous burst keeps the PE busy
           # through the preamble fill so the HAM clock-gate flips to 8/8
           # before the real stream starts (any >1.3us PE gap in the first
           # ~30us re-throttles it to half clock for 3.4us+)


def build_graph(bl=BL, s=S, e=E, d=D, ncores=NCORES):
    """Build the per-core Bass graph. All cores run the same graph (SPMD)."""
    G = e // 128        # e-chunks (contraction passes per tile)
    SW = 512            # s-window per context DMA (4 s-tiles)
    NSW = s // SW       # s-windows per batch
    TPW = SW // 128     # s-tiles per window
    NT = s // 128       # s-tiles per batch (scores columns)
    AF = mybir.ActivationFunctionType

    nc = bacc.Bacc("TRN2", target_bir_lowering=False, debug=False,
                   num_devices=ncores)

    # window-contiguous context: [b, w, p, g, s_local] flattened to
    # [b, w, 128, G*SW]; one window = 1 MiB fully contiguous.
    ctxw = nc.dram_tensor("ctxw", [bl, NSW, 128, G * SW], F16,
                          kind="ExternalInput")
    wcT = nc.dram_tensor("wcT", [128, G, d], F16, kind="ExternalInput")
    vb = nc.dram_tensor("vb", [128, d], F16, kind="ExternalInput")
    out = nc.dram_tensor("out", [128, bl * NT], F32, kind="ExternalOutput")

    ctx_r = ctxw.ap()

    with TileContext(nc) as tc:
        with (
            tc.tile_pool(name="const", bufs=1) as cpool,
            tc.tile_pool(name="ctx", bufs=6) as ctx_pool,
            tc.tile_pool(name="sim", bufs=4) as sim_pool,
            tc.tile_pool(name="prod", bufs=4) as prod_pool,
            tc.tile_pool(name="sc", bufs=2) as sc_pool,
            tc.tile_pool(name="small", bufs=2) as small_pool,
            tc.tile_pool(name="pc", bufs=7, space="PSUM") as pc_pool,
            tc.tile_pool(name="warm", bufs=1, space="PSUM") as warm_pool,
        ):
            # ---- constants / preamble ------------------------------------
            # DMA queue plan (start is latency-bound: each dma_start costs
            # ~0.65us of ring-sequencer time and ~2us of completion
            # latency):  wcT chunks split across the sync+scalar HWDGE
            # rings (preamble-only — the scalar queue must be DMA-free once
            # tanh work begins or a ring-full wait blocks the whole chain);
            # ctx0 chunks, ctx1 halves and ALL later windows ride the
            # gpsimd/SWDGE queue; outputs ride sync.
            junk = cpool.tile([128, 128], F16, tag="junk")
            nc.vector.memset(junk[:], 0.0)

            wct_sb = cpool.tile([128, G, d], F16, tag="wct")
            vb_sb = cpool.tile([128, d], F16, tag="vb")
            ctx0 = ctx_pool.tile([128, G * SW], F16, tag="ctx", name="ctx0")
            ctx1 = ctx_pool.tile([128, G * SW], F16, tag="ctx", name="ctx1")
            # wcT chunks split across the two HWDGE rings (preamble-only —
            # the scalar queue must be DMA-free before tanh work begins).
            # Staged just-in-time: only g0-g3 go up front; g4-g7 follow vb,
            # so the critical ctx0 chunk stream (gpsimd) isn't starved of
            # HBM bandwidth during the 7-12us fill window.
            for g in range(0, 4, 2):
                nc.sync.dma_start(out=wct_sb[:, g, :], in_=wcT.ap()[:, g, :])
                nc.scalar.dma_start(out=wct_sb[:, g + 1, :],
                                    in_=wcT.ap()[:, g + 1, :])
            nc.sync.dma_start(out=vb_sb[:], in_=vb.ap())
            for g in range(4, G, 2):
                nc.sync.dma_start(out=wct_sb[:, g, :], in_=wcT.ap()[:, g, :])
                nc.scalar.dma_start(out=wct_sb[:, g + 1, :],
                                    in_=wcT.ap()[:, g + 1, :])
            for g in range(G):
                nc.gpsimd.dma_start(out=ctx0[:, g * SW:(g + 1) * SW],
                                    in_=ctx_r[0, 0, :, g * SW:(g + 1) * SW])
            h = (G // 2) * SW
            nc.gpsimd.dma_start(out=ctx1[:, 0:h], in_=ctx_r[0, 1, :, 0:h])
            nc.gpsimd.dma_start(out=ctx1[:, h:G * SW],
                                in_=ctx_r[0, 1, :, h:G * SW])

            # PE warm-up: junk matmuls bridge the gap until chunk g0 lands;
            # real matmuls then run (briefly cold) while the HAM clock-gate
            # warms.
            warm_ps = warm_pool.tile([128, 128], F32, tag="warm")
            for _ in range(JUNK):
                nc.tensor.matmul(warm_ps[:, 0:64], lhsT=junk[:],
                                 rhs=junk[:, 0:64], start=True, stop=True)

            MUL = mybir.AluOpType.mult
            ADD = mybir.AluOpType.add

            def chain(pc, scores, st):
                # tanh -> fused (sim * v, free-axis sum) -> scores column
                sim = sim_pool.tile([128, 512], F16, tag="sim")
                prod = prod_pool.tile([128, 512], F16, tag="prod")
                nc.scalar.activation(sim[:], pc[:], AF.Tanh)
                nc.vector.tensor_mul(prod[:], sim[:], vb_sb[:])
                nc.vector.reduce_sum(scores[:, st:st + 1], prod[:],
                                     axis=mybir.AxisListType.X)

            # ---- main loop ------------------------------------------------
            # all batches' scores accumulate in one [128, bl*NT] tile so the
            # output leaves as one 62KB line-rate DMA overlapped under the
            # last window's compute, plus a 2KB tail DMA for the final 4
            # columns.
            sc_all = sc_pool.tile([128, bl * NT], F32, tag="scores")
            for b in range(bl):
                for sw in range(NSW):
                    if b == 0 and sw <= 1:
                        # pipe-fill: contraction swept g-outermost across
                        # all 4 window PSUM banks — each matmul needs only
                        # the (wcT, ctx) chunk pair g that has already
                        # arrived, so the PE computes through the fill.
                        src = ctx0 if sw == 0 else ctx1
                        pcs = [pc_pool.tile([128, 512], F32, tag="pc",
                                            name=f"pc{sw}_{t}")
                               for t in range(TPW)]
                        for g in range(G):
                            for t in range(TPW):
                                nc.tensor.matmul(
                                    pcs[t][:],
                                    lhsT=src[:, g * SW + t * 128:
                                             g * SW + (t + 1) * 128],
                                    rhs=wct_sb[:, g, :],
                                    start=(g == 0), stop=(g == G - 1),
                                )
                        for t in range(TPW):
                            chain(pcs[t], sc_all, b * NT + sw * TPW + t)
                        continue
                    ctx_t = ctx_pool.tile([128, G * SW], F16, tag="ctx")
                    # one contiguous 1MB fp16 read per window on the
                    # gpsimd/SWDGE queue
                    nc.gpsimd.dma_start(out=ctx_t[:], in_=ctx_r[b, sw])
                    for t in range(TPW):
                        st = b * NT + sw * TPW + t
                        if b == bl - 1 and sw == NSW - 1 and t == TPW - 1:
                            # final tile: two d-half matmul groups, so the
                            # first half's tanh/dot chain runs while the
                            # second half's matmuls are still on the PE.
                            s2 = small_pool.tile([128, 2], F32, tag="s2")
                            for q in range(2):
                                cut = slice(q * 256, (q + 1) * 256)
                                pch = pc_pool.tile([128, 512], F32,
                                                   tag="pc", name=f"pch{q}")
                                for g in range(G):
                                    nc.tensor.matmul(
                                        pch[:, cut],
                                        lhsT=ctx_t[:, g * SW + t * 128:
                                                   g * SW + (t + 1) * 128],
                                        rhs=wct_sb[:, g, cut],
                                        start=(g == 0), stop=(g == G - 1),
                                    )
                                sim = sim_pool.tile([128, 512], F16,
                                                    tag="sim",
                                                    name=f"simh{q}")
                                nc.scalar.activation(sim[:, cut],
                                                     pch[:, cut], AF.Tanh)
                                prod = prod_pool.tile([128, 512], F16,
                                                      tag="prod",
                                                      name=f"prodh{q}")
                                nc.vector.tensor_mul(prod[:, cut],
                                                     sim[:, cut],
                                                     vb_sb[:, cut])
                                nc.vector.reduce_sum(
                                    s2[:, q:q + 1], prod[:, cut],
                                    axis=mybir.AxisListType.X)
                            nc.vector.tensor_add(sc_all[:, st:st + 1],
                                                 s2[:, 0:1], s2[:, 1:2])
                            continue
                        pc = pc_pool.tile([128, 512], F32, tag="pc")
                        for g in range(G):
                            nc.tensor.matmul(
                                pc[:],
                                lhsT=ctx_t[:, g * SW + t * 128:
                                           g * SW + (t + 1) * 128],
                                rhs=wct_sb[:, g, :],
                                start=(g == 0), stop=(g == G - 1),
                            )
                        chain(pc, sc_all, st)
                    if b == bl - 1 and sw == NSW - 2:
                        # all but the final window's columns ride out under
                        # the last window's compute
                        cc = bl * NT - TPW
                        nc.sync.dma_start(out=out.ap()[:, 0:cc],
                                          in_=sc_all[:, 0:cc])
            # final 4 columns: the only output on the critical tail (2KB).
            # On the sync queue (scalar carries no DMAs; sync is idle now).
            cc = bl * NT - TPW
            nc.sync.dma_start(out=out.ap()[:, cc:bl * NT],
                              in_=sc_all[:, cc:bl * NT])

    nc.compile()
    return nc


def shard_inputs(hidden, context, mask, W_attn, b_attn, v,
                 bl=BL, s=S, e=E, d=D, ncores=NCORES):
    """Host-side shard + layout prep. Returns in_maps for run_bass_kernel_spmd."""
    G = e // 128
    SW = 512
    NSW = s // SW
    Wh = W_attn[:, :d].astype(np.float64)
    Wc = W_attn[:, d:].astype(np.float64)
    # Fold h_proj into the context (exact): delta_b = Wc^T (Wc Wc^T)^-1 hp_b
    # gives Wc (x + delta_b) = c_proj + h_proj_b.
    hp = hidden.astype(np.float64) @ Wh.T + b_attn.astype(np.float64)  # [B, d]
    alpha = np.linalg.solve(Wc @ Wc.T, hp.T)                           # [d, B]
    delta = (Wc.T @ alpha).T.astype(np.float32)                        # [B, e]

    # wcT[p, g, :] = Wc[:, g*128+p]  (moving operand rows = e within chunk)
    wcT = np.ascontiguousarray(
        np.asarray(W_attn[:, d:]).T.reshape(G, 128, d).transpose(1, 0, 2)
    ).astype(np.float16)
    vbc = np.ascontiguousarray(
        np.broadcast_to(v.astype(np.float16), (128, d)))

    in_maps = []
    for i in range(ncores):
        sl = slice(i * bl, (i + 1) * bl)
        # ctxw[b, w, p, g, s_l] = (ctx + delta)[b, w*SW+s_l, g*128+p]
        ctmp = (context[sl] + delta[sl][:, None, :]).astype(np.float16)
        ctxw = np.ascontiguousarray(
            ctmp.reshape(bl, NSW, SW, G, 128).transpose(0, 1, 4, 3, 2)
        ).reshape(bl, NSW, 128, G * SW)
        in_maps.append({
            "ctxw": ctxw,
            "wcT": wcT,
            "vb": vbc,
        })
    return in_maps


def postprocess(results, mask, bl=BL, s=S):
    """Gather per-core raw score tiles and finish mask+softmax on host."""
    NT = s // 128
    # per-core out [128, bl*NT]: column b*NT + t, partition p -> s = t*128+p
    sc = np.stack([r["out"] for r in results])           # [ncores, 128, bl*NT]
    sc = sc.reshape(-1, 128, bl, NT).transpose(0, 2, 3, 1)  # [nc, bl, NT, 128]
    scores = sc.reshape(-1, NT * 128)
    scores = scores.astype(np.float64)
    scores = np.where(np.asarray(mask) == 0, -np.inf, scores)
    mx = scores.max(axis=1, keepdims=True)
    ex = np.exp(scores - mx)
    return (ex / ex.sum(axis=1, keepdims=True)).astype(np.float32)


_CACHE = {}


def _ensure_ntff_hook_importable():
    """bass_utils' axon trace path imports antenv.axon_hooks, which this
    container's antenv stub lacks. Provide it (with the real ctypes hook when
    available) so BASS_TRACE=1 in the environment can't crash the run."""
    import sys as _sys
    import types as _types

    try:
        import antenv.axon_hooks  # noqa: F401
        return
    except ImportError:
        pass
    mod = _types.ModuleType("antenv.axon_hooks")
    mod._hook = None
    mod.set_axon_ntff_profile_hook = lambda h: setattr(mod, "_hook", h)
    mod.get_axon_ntff_profile_hook = lambda: mod._hook
    _sys.modules["antenv.axon_hooks"] = mod
    try:
        import antenv
        antenv.axon_hooks = mod
        from trn_agent_boot.trn_boot import _ntff_profile_via_ctypes
        mod._hook = _ntff_profile_via_ctypes("/opt/axon/libaxon_pjrt.so")
    except Exception:
        pass


def kernel(hidden, context, mask, W_attn, b_attn, v):
    _ensure_ntff_hook_importable()
    hidden = np.asarray(hidden, dtype=np.float32)
    context = np.asarray(context, dtype=np.float32)
    mask = np.asarray(mask)
    W_attn = np.asarray(W_attn, dtype=np.float32)
    b_attn = np.asarray(b_attn, dtype=np.float32)
    v = np.asarray(v, dtype=np.float32)
    if "nc" not in _CACHE:
        _CACHE["nc"] = build_graph()
    nc = _CACHE["nc"]
    in_maps = shard_inputs(hidden, context, mask, W_attn, b_attn, v)
    res = run_bass_kernel_spmd(nc, in_maps, core_ids=list(range(NCORES)))
    return postprocess(res.results, mask)
